# revision 12
# baseline (speedup 1.0000x reference)
"""Distributed Trainium2 kernel for the fused attention-autoencoder layer.

Reference math (per head h):
  Q = x @ Wq_h^T + bq_h ; K = x @ Wk_h^T + bk_h ; V = x @ Wv_h^T + bv_h
  scores = K^T Q / sqrt(E); A = softmax(scores, -1); Zh = V @ A
  O = concat_h(Zh) @ Wz^T + bz ; LN1 = ln(O)*g1+b1 + x
  FN = LN1 @ Wf^T + bf ; out = ln(FN)*g2+b2 + LN1

Restructuring (head h lives on core h):
  With xa = [x | 1] (augmented) and G~ = xa^T xa (symmetric):
    scores_h = Wka_h G~ Wqa_h^T / sqrt(E)  where Wka = [Wk|bk], Wqa = [Wq|bq]
  G~ is computed from 1024-row partials per core and AllReduced across
  4-core groups [[0,2,4,6],[1,3,5,7]] (2x the matmul work of an 8-way
  512-row split, but that work hides the collective-stream startup
  barrier, and the 4-way AR is cheaper than the 8-way).
  A_h = softmax(scores_h). Since the head-concat matmul distributes,
    O = sum_h V_h A_h Wz_h^T = x (sum_h C_h) + 1 (sum_h r_h)^T
  with C_h = Wv_h^T A'_h Wz_h^T and r_h = bv_h^T A'_h Wz_h^T + bz/8, so the
  cross-core exchange is a chunked 8-way AllReduce of the tiny [E+1, E]
  C/r matrix (NOT an S-sized ReduceScatter), and each core then runs one
  small Opart matmul over only its own S/8 rows: O_rows = x_rows C + 1 r^T,
  followed by the LN1/FFN/LN2 pipeline on those rows. The host concatenates
  the 8 contiguous row shards.
"""

import numpy as np
import ml_dtypes

import concourse.bass as bass
import concourse.mybir as mybir
import concourse.tile as tile
from concourse import bacc
from concourse.bass_utils import run_bass_kernel_spmd
from concourse.masks import make_identity

S, E, H = 4096, 1024, 8
P = 128
EA = 1152          # augmented (E + ones col) padded to 9*128
NET = E // P       # 8
NAT = EA // P      # 9
SS = S // H        # 512 rows per core (own contiguous output shard)
NST = SS // P      # 4
SG = 512           # rows per core for the G partial (8-way AR)
NSG = SG // P      # 4
NH = E // 512      # 2 free-dim halves
G_CHUNKS = [(0, 3), (3, 6), (6, 9)]  # G AllReduce row-tile chunks
EPS = 1e-5
SCALE = 1.0 / 32.0  # 1/sqrt(E)

F32 = mybir.dt.float32
BF16 = mybir.dt.bfloat16

LAST_RESULT = None  # test harness reads exec_time_ns off this


def _bcast_row(t: bass.AP) -> bass.AP:
    """[1, n] DRAM row -> partition-broadcast AP."""
    return bass.AP(tensor=t.tensor, offset=t.offset, ap=[[0, P], [1, t.shape[-1]]])


def build_nc(id_g1b1=False, id_g2b2=False):
    nc = bacc.Bacc(num_devices=H)

    xsa = nc.declare_dram_parameter("xsa", [SG, EA], BF16, isOutput=False)
    xaT = nc.declare_dram_parameter("xaT", [E, SS], BF16, isOutput=False)
    xs = nc.declare_dram_parameter("xs", [SS, E], F32, isOutput=False)
    wqa = nc.declare_dram_parameter("wqa", [EA, E], BF16, isOutput=False)
    wka = nc.declare_dram_parameter("wka", [EA, E], BF16, isOutput=False)
    wv = nc.declare_dram_parameter("wv", [E, E], BF16, isOutput=False)
    wzT = nc.declare_dram_parameter("wzT", [E, E], BF16, isOutput=False)
    wfT = nc.declare_dram_parameter("wfT", [E, E], BF16, isOutput=False)
    bv = nc.declare_dram_parameter("bv", [P, NET], BF16, isOutput=False)
    rows = nc.declare_dram_parameter("rows", [6, E], F32, isOutput=False)
    out = nc.declare_dram_parameter("out", [SS, E], F32, isOutput=True)

    g_part = nc.dram_tensor("g_part", [EA, EA], BF16)
    g_full = nc.dram_tensor("g_full", [EA, EA], BF16)
    c_part = nc.dram_tensor("c_part", [E + P, E], BF16)
    c_full = nc.dram_tensor("c_full", [E + P, E], BF16, addr_space="Shared")

    rg8 = [list(range(H))]

    both_id = id_g1b1 and id_g2b2
    NROWS = 1 if both_id else 5  # broadcast LN/FFN rows kept in SBUF
    # row indices within rows_bc
    if both_id:
        L_G1 = L_B1 = L_G2 = L_B2 = 0
        L_BF = 0
    else:
        L_G1, L_B1, L_BF, L_G2, L_B2 = range(5)

    def mm_loop(lhs_fn, rhs_fn, nk, evac, ps_pool):
        pss = [
            ps_pool.tile([P, 512], F32, tag="mm", name=f"psmm_{n}") for n in range(NH)
        ]
        for k in range(nk):
            lhs = lhs_fn(k)
            for n in range(NH):
                nc.tensor.matmul(
                    pss[n], lhs, rhs_fn(k, n), start=(k == 0), stop=(k == nk - 1)
                )
        for n in range(NH):
            evac(n, pss[n])

    with tile.TileContext(nc) as tc:
        with (
            tc.tile_pool(name="singles", bufs=1) as singles,
            tc.tile_pool(name="stat", bufs=4) as stat,
            tc.tile_pool(name="ps_mm", bufs=6, space="PSUM") as ps_mm,
            tc.tile_pool(name="ps_tr", bufs=2, space="PSUM") as ps_tr,
        ):
            ident = singles.tile([P, P], BF16)
            identf = singles.tile([P, P], F32)
            bz8_sb = singles.tile([1, E], F32)
            bv_sb = singles.tile([P, NET], BF16)
            rcp_sb = singles.tile([P, NET], F32)
            rbc_sb = singles.tile([P, E], BF16)
            eps_sb = singles.tile([P, 1], F32)

            with tc.tile_pool(name="pc", bufs=1) as pc, \
                 tc.tile_pool(name="p5", bufs=3) as p5:
                cs_sb = pc.tile([P, NET, E], BF16)   # AllReduced sum_h C_h
                xaT_sb = pc.tile([P, NET, SS], BF16)
                rows_bc = pc.tile([P, NROWS, E], F32)
                with tc.tile_pool(name="pwz", bufs=1) as pwz:
                    wv_sb = pwz.tile([P, NET, E], BF16)
                    wzT_sb = pwz.tile([P, NET, E], BF16)
                    with tc.tile_pool(name="pb", bufs=1) as pb:
                        b_sb = pb.tile([P, NET, E], BF16)
                        with tc.tile_pool(name="pat", bufs=1) as pat:
                            at_sb = pat.tile([P, NET, E], BF16)
                            with tc.tile_pool(name="pwqk", bufs=1) as pwqk:
                                wqa_sb = pwqk.tile([P, NAT, E], BF16)
                                wka_sb = pwqk.tile([P, NAT, E], BF16)
                                u_sb = pwqk.tile([P, NAT, E], BF16)
                                with tc.tile_pool(name="pxsa", bufs=1) as pxsa, \
                                     tc.tile_pool(name="p1w", bufs=2) as p1w:
                                    # ===== phase 1: G~ partial (1024 rows) +
                                    # chunked 4-way AR =====
                                    xsa_sb = pxsa.tile([P, NSG, EA], BF16)
                                    hs = NSG // 2
                                    nc.sync.dma_start(
                                        out=xsa_sb[:, 0:hs, :],
                                        in_=xsa[0 : hs * P, :]
                                        .rearrange("(t p) e -> p t e", p=P),
                                    )
                                    nc.sync.dma_start(
                                        out=xsa_sb[:, hs:NSG, :],
                                        in_=xsa[hs * P : SG, :]
                                        .rearrange("(t p) e -> p t e", p=P),
                                    )
                                    nchunks = [(0, 512), (512, 512), (1024, EA - 1024)]
                                    for (t0, t1) in G_CHUNKS:
                                        for m in range(t0, t1):
                                            gp = p1w.tile([P, EA], BF16, tag="gp")
                                            for (n0, nw) in nchunks:
                                                ps = ps_mm.tile(
                                                    [P, nw], F32, tag="mm", name="psg"
                                                )
                                                for k in range(NSG):
                                                    nc.tensor.matmul(
                                                        ps,
                                                        xsa_sb[:, k, m * P : (m + 1) * P],
                                                        xsa_sb[:, k, n0 : n0 + nw],
                                                        start=(k == 0),
                                                        stop=(k == NSG - 1),
                                                    )
                                                nc.vector.tensor_copy(
                                                    out=gp[:, n0 : n0 + nw], in_=ps
                                                )
                                            nc.sync.dma_start(
                                                out=g_part[m * P : (m + 1) * P, :], in_=gp
                                            )
                                        r1 = min(t1 * P, E + 1)
                                        nc.gpsimd.collective_compute(
                                            "AllReduce",
                                            mybir.AluOpType.add,
                                            replica_groups=rg8,
                                            ins=[g_part[t0 * P : r1, :]],
                                            outs=[g_full[t0 * P : r1, :]],
                                        )

                                    # ---- constants / weights (emitted after
                                    # the collectives: G path wins DMA prio) ----
                                    make_identity(nc, ident)
                                    make_identity(nc, identf)
                                    nc.sync.dma_start(out=bz8_sb, in_=rows[0:1, :])
                                    nc.sync.dma_start(out=bv_sb, in_=bv[:, :])
                                    nc.vector.memset(eps_sb, EPS)
                                    nc.sync.dma_start(
                                        out=wqa_sb,
                                        in_=wqa[:, :].rearrange("(t p) e -> p t e", p=P),
                                    )
                                    nc.sync.dma_start(
                                        out=wka_sb,
                                        in_=wka[:, :].rearrange("(t p) e -> p t e", p=P),
                                    )
                                    nc.sync.dma_start(
                                        out=wv_sb,
                                        in_=wv[:, :].rearrange("(t p) e -> p t e", p=P),
                                    )
                                    nc.sync.dma_start(
                                        out=wzT_sb,
                                        in_=wzT[:, :].rearrange("(t p) e -> p t e", p=P),
                                    )
                                    # prefetch tail-phase inputs now; they
                                    # land long before the C AllReduce
                                    nc.sync.dma_start(
                                        out=xaT_sb,
                                        in_=xaT[:, :].rearrange("(t p) s -> p t s", p=P),
                                    )
                                    if both_id:
                                        nc.sync.dma_start(
                                            out=rows_bc[:, 0, :],
                                            in_=_bcast_row(rows[3:4, :]),
                                        )
                                    else:
                                        for k in range(5):
                                            nc.sync.dma_start(
                                                out=rows_bc[:, k, :],
                                                in_=_bcast_row(rows[k + 1 : k + 2, :]),
                                            )

                                # ===== phase 2: U = G~ @ wqa, overlapping
                                # the chunked AR (psum persists per chunk)
                                with tc.tile_pool(name="pg", bufs=1) as pg:
                                    g_sb = pg.tile([P, NAT, EA], BF16)
                                    nc.vector.memset(g_sb[:, NET, :], 0.0)
                                    for (t0, t1) in G_CHUNKS:
                                        tf = min(t1, NET)  # full 128-row tiles
                                        nc.sync.dma_start(
                                            out=g_sb[:, t0:tf, :],
                                            in_=g_full[t0 * P : tf * P, :]
                                            .rearrange("(t p) e -> p t e", p=P),
                                        )
                                        if t1 > NET:
                                            nc.sync.dma_start(
                                                out=g_sb[0:1, NET, :],
                                                in_=g_full[E : E + 1, :],
                                            )
                                    for (m0, m1) in [(0, 3), (3, 6), (6, 9)]:
                                        pss = {}
                                        for m in range(m0, m1):
                                            for n in range(NH):
                                                pss[m, n] = ps_mm.tile(
                                                    [P, 512], F32, tag="mm",
                                                    name=f"psu_{m}_{n}",
                                                )
                                        for (t0, t1) in G_CHUNKS:
                                            for m in range(m0, m1):
                                                for n in range(NH):
                                                    for k in range(t0, t1):
                                                        nc.tensor.matmul(
                                                            pss[m, n],
                                                            g_sb[:, k, m * P : (m + 1) * P],
                                                            wqa_sb[:, k, n * 512 : (n + 1) * 512],
                                                            start=(k == 0),
                                                            stop=(k == NAT - 1),
                                                        )
                                        for m in range(m0, m1):
                                            for n in range(NH):
                                                nc.vector.tensor_copy(
                                                    out=u_sb[:, m, n * 512 : (n + 1) * 512],
                                                    in_=pss[m, n],
                                                )

                                # ===== phase 3: scores + softmax + A^T =====
                                with tc.tile_pool(name="p3", bufs=3) as p3:
                                    for m in range(NET):
                                        pss = [
                                            ps_mm.tile([P, 512], F32, tag="mm",
                                                       name=f"pssc_{n}")
                                            for n in range(NH)
                                        ]
                                        for k in range(NAT):
                                            lhs = wka_sb[:, k, m * P : (m + 1) * P]
                                            for n in range(NH):
                                                nc.tensor.matmul(
                                                    pss[n], lhs,
                                                    u_sb[:, k, n * 512 : (n + 1) * 512],
                                                    start=(k == 0), stop=(k == NAT - 1),
                                                )
                                        mxs = stat.tile([P, NH], F32, tag="mxs")
                                        for n in range(NH):
                                            nc.vector.reduce_max(
                                                out=mxs[:, n : n + 1], in_=pss[n],
                                                axis=mybir.AxisListType.X,
                                            )
                                        mx = stat.tile([P, 1], F32, tag="mx")
                                        nc.vector.tensor_max(
                                            mx, mxs[:, 0:1], mxs[:, 1:2]
                                        )
                                        negmx = stat.tile([P, 1], F32, tag="negmx")
                                        nc.vector.tensor_scalar_mul(negmx, mx, -SCALE)
                                        a_bf = p3.tile([P, E], BF16, tag="abf")
                                        rsums = stat.tile([P, NH], F32, tag="rsums")
                                        for n in range(NH):
                                            nc.scalar.activation(
                                                out=a_bf[:, n * 512 : (n + 1) * 512],
                                                in_=pss[n],
                                                func=mybir.ActivationFunctionType.Exp,
                                                bias=negmx, scale=SCALE,
                                                accum_out=rsums[:, n : n + 1],
                                            )
                                        rsum = stat.tile([P, 1], F32, tag="rsum")
                                        nc.vector.tensor_add(
                                            rsum, rsums[:, 0:1], rsums[:, 1:2]
                                        )
                                        nc.vector.reciprocal(
                                            out=rcp_sb[:, m : m + 1], in_=rsum
                                        )
                                        for fb in range(NET):
                                            pst = ps_tr.tile([P, P], BF16, tag="tr", name="pst")
                                            nc.tensor.transpose(
                                                pst, a_bf[:, fb * P : (fb + 1) * P], ident
                                            )
                                            nc.vector.tensor_copy(
                                                out=at_sb[:, fb, m * P : (m + 1) * P],
                                                in_=pst,
                                            )

                            # ===== phase 4a: B = AT.T @ WzT (row-scaled) =====
                            for m in range(NET):
                                mm_loop(
                                    lambda k: at_sb[:, k, m * P : (m + 1) * P],
                                    lambda k, n: wzT_sb[:, k, n * 512 : (n + 1) * 512],
                                    NET,
                                    lambda n, ps: nc.vector.tensor_scalar_mul(
                                        b_sb[:, m, n * 512 : (n + 1) * 512],
                                        ps,
                                        rcp_sb[:, m : m + 1],
                                    ),
                                    ps_mm,
                                )

                        # ===== phase 4b: r + C_h = Wv^T B per row-tile, DMA
                        # out, chunked 8-way AllReduce of [C; r] =====
                        # r = bv^T B + bz/8 -> row E of the AR payload
                        r_sb = stat.tile([1, E], F32, tag="rrow")
                        rbf = p5.tile([1, E], BF16, tag="rbf")
                        for n in range(NH):
                            psr = ps_mm.tile([1, 512], F32, tag="mm", name="psr")
                            for k in range(NET):
                                nc.tensor.matmul(
                                    psr,
                                    bv_sb[:, k : k + 1],
                                    b_sb[:, k, n * 512 : (n + 1) * 512],
                                    start=(k == 0),
                                    stop=(k == NET - 1),
                                )
                            nc.vector.tensor_add(
                                r_sb[:, n * 512 : (n + 1) * 512],
                                psr,
                                bz8_sb[:, n * 512 : (n + 1) * 512],
                            )
                        nc.vector.tensor_copy(out=rbf, in_=r_sb)
                        nc.sync.dma_start(out=c_part[E : E + 1, :], in_=rbf)
                        for m in range(NET):
                            cstg = p5.tile([P, E], BF16, tag="cstg", bufs=3)
                            mm_loop(
                                lambda k: wv_sb[:, k, m * P : (m + 1) * P],
                                lambda k, n: b_sb[:, k, n * 512 : (n + 1) * 512],
                                NET,
                                lambda n, ps: nc.vector.tensor_copy(
                                    out=cstg[:, n * 512 : (n + 1) * 512], in_=ps
                                ),
                                ps_mm,
                            )
                            nc.sync.dma_start(
                                out=c_part[m * P : (m + 1) * P, :], in_=cstg
                            )
                            if m == 3:
                                nc.gpsimd.collective_compute(
                                    "AllReduce",
                                    mybir.AluOpType.add,
                                    replica_groups=rg8,
                                    ins=[c_part[0:512, :]],
                                    outs=[c_full[0:512, :]],
                                )
                        nc.gpsimd.collective_compute(
                            "AllReduce",
                            mybir.AluOpType.add,
                            replica_groups=rg8,
                            ins=[c_part[512 : E + 1, :]],
                            outs=[c_full[512 : E + 1, :]],
                        )

                # ===== phase 5: Opart on own SS rows (chunk-overlapped with
                # the C AllReduce), then LN1+transposes, then FFN+LN2 =====
                with tc.tile_pool(name="pln", bufs=1) as pln, \
                     tc.tile_pool(name="p7", bufs=3) as p7:
                    osb = pln.tile([P, NST, E], BF16)
                    ln1_sb = pln.tile([P, NST, E], F32)
                    l1t_sb = pln.tile([P, NET, SS], BF16)
                    wfT_sb = pln.tile([P, NET, E], BF16)
                    nc.sync.dma_start(
                        out=wfT_sb,
                        in_=wfT[:, :].rearrange("(t p) e -> p t e", p=P),
                    )

                    # residual rows: prefetch all 4 tiles now
                    xsts = []
                    for st in range(NST):
                        xst = p7.tile([P, E], F32, tag="xst", bufs=4)
                        nc.sync.dma_start(out=xst, in_=xs[st * P : (st + 1) * P, :])
                        xsts.append(xst)

                    # AllReduced C chunks -> SBUF; r row broadcast
                    nc.sync.dma_start(
                        out=cs_sb[:, 0:4, :],
                        in_=c_full[0:512, :].rearrange("(t p) e -> p t e", p=P),
                    )
                    nc.sync.dma_start(
                        out=cs_sb[:, 4:8, :],
                        in_=c_full[512:E, :].rearrange("(t p) e -> p t e", p=P),
                    )
                    nc.sync.dma_start(out=rbc_sb, in_=_bcast_row(c_full[E : E + 1, :]))

                    # Opart: 2 groups of 2 m-tiles; k accumulation ordered by
                    # AR chunk so group 1 starts once chunk 1 has landed
                    for (m0, m1) in [(0, 2), (2, 4)]:
                        pss = {}
                        for m in range(m0, m1):
                            for n in range(NH):
                                pss[m, n] = ps_mm.tile(
                                    [P, 512], F32, tag="mm", name=f"pso_{m}_{n}"
                                )
                        for (k0, k1) in [(0, 4), (4, 8)]:
                            for m in range(m0, m1):
                                for n in range(NH):
                                    for k in range(k0, k1):
                                        nc.tensor.matmul(
                                            pss[m, n],
                                            xaT_sb[:, k, m * P : (m + 1) * P],
                                            cs_sb[:, k, n * 512 : (n + 1) * 512],
                                            start=(k == 0),
                                            stop=(k == NET - 1),
                                        )
                        for m in range(m0, m1):
                            for n in range(NH):
                                nc.vector.tensor_add(
                                    osb[:, m, n * 512 : (n + 1) * 512],
                                    pss[m, n],
                                    rbc_sb[:, n * 512 : (n + 1) * 512],
                                )

                    def layer_norm(dst, src, r_g, r_b, skip_gb):
                        bst = stat.tile([P, 2, 6], F32, tag="bst")
                        nc.vector.bn_stats(out=bst[:, 0, :], in_=src[:, 0:512])
                        nc.vector.bn_stats(out=bst[:, 1, :], in_=src[:, 512:E])
                        mv = stat.tile([P, 2], F32, tag="mv")
                        nc.vector.bn_aggr(out=mv, in_=bst)
                        sd = stat.tile([P, 1], F32, tag="sd")
                        nc.scalar.activation(
                            out=sd, in_=mv[:, 1:2],
                            func=mybir.ActivationFunctionType.Sqrt, bias=eps_sb[:, :],
                        )
                        rstd = stat.tile([P, 1], F32, tag="rstd")
                        nc.vector.reciprocal(out=rstd, in_=sd)
                        nc.vector.tensor_scalar(
                            out=dst, in0=src, scalar1=mv[:, 0:1], scalar2=rstd,
                            op0=mybir.AluOpType.subtract, op1=mybir.AluOpType.mult,
                        )
                        if not skip_gb:
                            nc.vector.tensor_mul(dst, dst, rows_bc[:, r_g, :])
                            nc.vector.tensor_add(dst, dst, rows_bc[:, r_b, :])

                    # LN1 (+x residual) and transposes for ALL row tiles first
                    # so the FFN/LN2 loop below pipelines PE vs DVE cleanly
                    for st in range(NST):
                        t1 = ln1_sb[:, st, :]
                        ln = p7.tile([P, E], F32, tag="ln")
                        layer_norm(ln, osb[:, st, :], L_G1, L_B1, id_g1b1)
                        nc.vector.tensor_add(t1, ln, xsts[st])
                    for st in range(NST):
                        t1 = ln1_sb[:, st, :]
                        for eb in range(NET):
                            pstf = ps_tr.tile([P, P], F32, tag="tr", name="pstf")
                            nc.tensor.transpose(pstf, t1[:, eb * P : (eb + 1) * P], identf)
                            nc.scalar.activation(
                                out=l1t_sb[:, eb, st * P : (st + 1) * P],
                                in_=pstf,
                                func=mybir.ActivationFunctionType.Copy,
                            )
                    # FFN + LN2 per row tile
                    for st in range(NST):
                        f1 = p7.tile([P, E], F32, tag="f1")
                        mm_loop(
                            lambda k: l1t_sb[:, k, st * P : (st + 1) * P],
                            lambda k, n: wfT_sb[:, k, n * 512 : (n + 1) * 512],
                            NET,
                            lambda n, ps: nc.vector.tensor_add(
                                f1[:, n * 512 : (n + 1) * 512],
                                ps,
                                rows_bc[:, L_BF, n * 512 : (n + 1) * 512],
                            ),
                            ps_mm,
                        )
                        ln2 = p7.tile([P, E], F32, tag="ln2")
                        layer_norm(ln2, f1, L_G2, L_B2, id_g2b2)
                        fo = p7.tile([P, E], F32, tag="ln")
                        nc.vector.tensor_add(fo, ln2, ln1_sb[:, st, :])
                        nc.sync.dma_start(out=out[st * P : (st + 1) * P, :], in_=fo)

    nc.finalize()
    return nc


_NC_CACHE = None


def kernel(**inputs) -> np.ndarray:
    global _NC_CACHE, LAST_RESULT
    x = np.asarray(inputs["x"], np.float32)
    Wq = np.asarray(inputs["Wq"], np.float32)
    bq = np.asarray(inputs["bq"], np.float32)
    Wk = np.asarray(inputs["Wk"], np.float32)
    bk = np.asarray(inputs["bk"], np.float32)
    Wv = np.asarray(inputs["Wv"], np.float32)
    bv = np.asarray(inputs["bv"], np.float32)
    Wz = np.asarray(inputs["Wz"], np.float32)
    bz = np.asarray(inputs["bz"], np.float32)
    g1 = np.asarray(inputs["g1"], np.float32)
    b1 = np.asarray(inputs["b1"], np.float32)
    Wf = np.asarray(inputs["Wf"], np.float32)
    bf_ = np.asarray(inputs["bf"], np.float32)
    g2 = np.asarray(inputs["g2"], np.float32)
    b2 = np.asarray(inputs["b2"], np.float32)

    BF = ml_dtypes.bfloat16
    id_g1b1 = bool(np.all(g1 == 1.0) and np.all(b1 == 0.0))
    id_g2b2 = bool(np.all(g2 == 1.0) and np.all(b2 == 0.0))
    key = (id_g1b1, id_g2b2)
    if _NC_CACHE is None or _NC_CACHE[0] != key:
        _NC_CACHE = (key, build_nc(id_g1b1, id_g2b2))
    nc = _NC_CACHE[1]

    wfT_np = np.ascontiguousarray(Wf.T).astype(BF)
    rows_np = np.ascontiguousarray(
        np.stack([bz / H, g1, b1, bf_, g2, b2]).astype(np.float32)
    )
    pad_w = np.zeros((EA - E - 1, E), np.float32)

    in_maps = []
    for h in range(H):
        own = slice(h * SS, (h + 1) * SS)      # own output shard rows
        xga = x[h * SG : (h + 1) * SG]         # G-partial rows (8-way AR)
        xsa_h = np.concatenate(
            [xga, np.ones((SG, 1), np.float32), np.zeros((SG, EA - E - 1), np.float32)],
            axis=1,
        ).astype(BF)
        xaT_h = np.ascontiguousarray(x[own].T).astype(BF)
        xs_h = np.ascontiguousarray(x[own])
        wqa_h = np.concatenate([Wq[h].T, bq[h][None, :], pad_w], axis=0).astype(BF)
        wka_h = np.concatenate([Wk[h].T, bk[h][None, :], pad_w], axis=0).astype(BF)
        wzT_h = np.ascontiguousarray(Wz[:, h * E : (h + 1) * E].T).astype(BF)
        bv_h = np.ascontiguousarray(bv[h].reshape(NET, P).T).astype(BF)
        in_maps.append(
            {
                "xsa": np.ascontiguousarray(xsa_h),
                "xaT": xaT_h,
                "xs": xs_h,
                "wqa": np.ascontiguousarray(wqa_h),
                "wka": np.ascontiguousarray(wka_h),
                "wv": Wv[h].astype(BF),
                "wzT": wzT_h,
                "wfT": wfT_np,
                "bv": bv_h,
                "rows": rows_np,
            }
        )

    res = run_bass_kernel_spmd(nc, in_maps, list(range(H)))
    LAST_RESULT = res
    out = np.empty((S, E), np.float32)
    for h in range(H):
        out[h * SS : (h + 1) * SS] = res.results[h]["out"]
    return out


# revision 15
# speedup vs baseline: 1.2011x; 1.2011x over previous
"""Distributed Trainium2 kernel for the fused attention-autoencoder layer.

Reference math (per head h):
  Q = x @ Wq_h^T + bq_h ; K = x @ Wk_h^T + bk_h ; V = x @ Wv_h^T + bv_h
  scores = K^T Q / sqrt(E); A = softmax(scores, -1); Zh = V @ A
  O = concat_h(Zh) @ Wz^T + bz ; LN1 = ln(O)*g1+b1 + x
  FN = LN1 @ Wf^T + bf ; out = ln(FN)*g2+b2 + LN1

Restructuring (head h lives on core h):
  With xa = [x | 1] (augmented) and G~ = xa^T xa (symmetric):
    scores_h = Wka_h G~ Wqa_h^T / sqrt(E)  where Wka = [Wk|bk], Wqa = [Wq|bq]
  G~ is computed from 1024-row partials per core and AllReduced across
  4-core groups [[0,2,4,6],[1,3,5,7]] (2x the matmul work of an 8-way
  512-row split, but that work hides the collective-stream startup
  barrier, and the 4-way AR is cheaper than the 8-way).
  A_h = softmax(scores_h). Since the head-concat matmul distributes,
    O = sum_h V_h A_h Wz_h^T = x (sum_h C_h) + 1 (sum_h r_h)^T
  with C_h = Wv_h^T A'_h Wz_h^T and r_h = bv_h^T A'_h Wz_h^T + bz/8, so the
  cross-core exchange is a chunked 8-way AllReduce of the tiny [E+1, E]
  C/r matrix (NOT an S-sized ReduceScatter), and each core then runs one
  small Opart matmul over only its own S/8 rows: O_rows = x_rows C + 1 r^T,
  followed by the LN1/FFN/LN2 pipeline on those rows. The host concatenates
  the 8 contiguous row shards.
"""

import numpy as np
import ml_dtypes

import concourse.bass as bass
import concourse.mybir as mybir
import concourse.tile as tile
from concourse import bacc
from concourse.bass_utils import run_bass_kernel_spmd
from concourse.masks import make_identity

S, E, H = 4096, 1024, 8
P = 128
EA = 1152          # augmented (E + ones col) padded to 9*128
NET = E // P       # 8
NAT = EA // P      # 9
SS = S // H        # 512 rows per core (own contiguous output shard)
NST = SS // P      # 4
SG = 512           # rows per core for the G partial (8-way AR)
NSG = SG // P      # 4
NH = E // 512      # 2 free-dim halves
G_CHUNKS = [(0, 9)]  # single G AllReduce (per-op fixed cost dominates)
EPS = 1e-5
SCALE = 1.0 / 32.0  # 1/sqrt(E)

F32 = mybir.dt.float32
BF16 = mybir.dt.bfloat16

LAST_RESULT = None  # test harness reads exec_time_ns off this


def _bcast_row(t: bass.AP) -> bass.AP:
    """[1, n] DRAM row -> partition-broadcast AP."""
    return bass.AP(tensor=t.tensor, offset=t.offset, ap=[[0, P], [1, t.shape[-1]]])


def build_nc(id_g1b1=False, id_g2b2=False):
    nc = bacc.Bacc(num_devices=H)

    xsa = nc.declare_dram_parameter("xsa", [SG, EA], BF16, isOutput=False)
    xaT = nc.declare_dram_parameter("xaT", [E, SS], BF16, isOutput=False)
    xs = nc.declare_dram_parameter("xs", [SS, E], F32, isOutput=False)
    wqa = nc.declare_dram_parameter("wqa", [EA, E], BF16, isOutput=False)
    wka = nc.declare_dram_parameter("wka", [EA, E], BF16, isOutput=False)
    wv = nc.declare_dram_parameter("wv", [E, E], BF16, isOutput=False)
    wzT = nc.declare_dram_parameter("wzT", [E, E], BF16, isOutput=False)
    wfT = nc.declare_dram_parameter("wfT", [E, E], BF16, isOutput=False)
    bv = nc.declare_dram_parameter("bv", [P, NET], BF16, isOutput=False)
    rows = nc.declare_dram_parameter("rows", [6, E], F32, isOutput=False)
    out = nc.declare_dram_parameter("out", [SS, E], F32, isOutput=True)

    g_part = nc.dram_tensor("g_part", [EA, EA], BF16)
    g_full = nc.dram_tensor("g_full", [EA, EA], BF16)
    c_part = nc.dram_tensor("c_part", [E + P, E], BF16)
    c_full = nc.dram_tensor("c_full", [E + P, E], BF16, addr_space="Shared")

    rg8 = [list(range(H))]

    both_id = id_g1b1 and id_g2b2
    NROWS = 1 if both_id else 5  # broadcast LN/FFN rows kept in SBUF
    # row indices within rows_bc
    if both_id:
        L_G1 = L_B1 = L_G2 = L_B2 = 0
        L_BF = 0
    else:
        L_G1, L_B1, L_BF, L_G2, L_B2 = range(5)

    def mm_loop(lhs_fn, rhs_fn, nk, evac, ps_pool):
        pss = [
            ps_pool.tile([P, 512], F32, tag="mm", name=f"psmm_{n}") for n in range(NH)
        ]
        for k in range(nk):
            lhs = lhs_fn(k)
            for n in range(NH):
                nc.tensor.matmul(
                    pss[n], lhs, rhs_fn(k, n), start=(k == 0), stop=(k == nk - 1)
                )
        for n in range(NH):
            evac(n, pss[n])

    with tile.TileContext(nc) as tc:
        with (
            tc.tile_pool(name="singles", bufs=1) as singles,
            tc.tile_pool(name="stat", bufs=4) as stat,
            tc.tile_pool(name="ps_mm", bufs=6, space="PSUM") as ps_mm,
            tc.tile_pool(name="ps_tr", bufs=2, space="PSUM") as ps_tr,
        ):
            ident = singles.tile([P, P], BF16)
            identf = singles.tile([P, P], F32)
            bz8_sb = singles.tile([1, E], F32)
            bv_sb = singles.tile([P, NET], BF16)
            rcp_sb = singles.tile([P, NET], F32)
            rbc_sb = singles.tile([P, E], BF16)
            eps_sb = singles.tile([P, 1], F32)

            with tc.tile_pool(name="pc", bufs=1) as pc, \
                 tc.tile_pool(name="p5", bufs=3) as p5:
                cs_sb = pc.tile([P, NET, E], BF16)   # AllReduced sum_h C_h
                xaT_sb = pc.tile([P, NET, SS], BF16)
                rows_bc = pc.tile([P, NROWS, E], F32)
                with tc.tile_pool(name="pwz", bufs=1) as pwz:
                    wv_sb = pwz.tile([P, NET, E], BF16)
                    wzT_sb = pwz.tile([P, NET, E], BF16)
                    with tc.tile_pool(name="pb", bufs=1) as pb:
                        b_sb = pb.tile([P, NET, E], BF16)
                        with tc.tile_pool(name="pat", bufs=1) as pat:
                            at_sb = pat.tile([P, NET, E], BF16)
                            with tc.tile_pool(name="pwqk", bufs=1) as pwqk:
                                wqa_sb = pwqk.tile([P, NAT, E], BF16)
                                wka_sb = pwqk.tile([P, NAT, E], BF16)
                                u_sb = pwqk.tile([P, NAT, E], BF16)
                                with tc.tile_pool(name="pxsa", bufs=1) as pxsa, \
                                     tc.tile_pool(name="p1w", bufs=2) as p1w:
                                    # ===== phase 1: G~ partial (1024 rows) +
                                    # chunked 4-way AR =====
                                    xsa_sb = pxsa.tile([P, NSG, EA], BF16)
                                    hs = NSG // 2
                                    nc.sync.dma_start(
                                        out=xsa_sb[:, 0:hs, :],
                                        in_=xsa[0 : hs * P, :]
                                        .rearrange("(t p) e -> p t e", p=P),
                                    )
                                    nc.sync.dma_start(
                                        out=xsa_sb[:, hs:NSG, :],
                                        in_=xsa[hs * P : SG, :]
                                        .rearrange("(t p) e -> p t e", p=P),
                                    )
                                    nchunks = [(0, 512), (512, 512), (1024, EA - 1024)]
                                    for (t0, t1) in G_CHUNKS:
                                        for m in range(t0, t1):
                                            gp = p1w.tile([P, EA], BF16, tag="gp")
                                            for (n0, nw) in nchunks:
                                                ps = ps_mm.tile(
                                                    [P, nw], F32, tag="mm", name="psg"
                                                )
                                                for k in range(NSG):
                                                    nc.tensor.matmul(
                                                        ps,
                                                        xsa_sb[:, k, m * P : (m + 1) * P],
                                                        xsa_sb[:, k, n0 : n0 + nw],
                                                        start=(k == 0),
                                                        stop=(k == NSG - 1),
                                                    )
                                                nc.vector.tensor_copy(
                                                    out=gp[:, n0 : n0 + nw], in_=ps
                                                )
                                            nc.sync.dma_start(
                                                out=g_part[m * P : (m + 1) * P, :], in_=gp
                                            )
                                        r1 = min(t1 * P, E + 1)
                                        nc.gpsimd.collective_compute(
                                            "AllReduce",
                                            mybir.AluOpType.add,
                                            replica_groups=rg8,
                                            ins=[g_part[t0 * P : r1, :]],
                                            outs=[g_full[t0 * P : r1, :]],
                                        )

                                    # ---- constants / weights (emitted after
                                    # the collectives: G path wins DMA prio) ----
                                    make_identity(nc, ident)
                                    make_identity(nc, identf)
                                    nc.sync.dma_start(out=bz8_sb, in_=rows[0:1, :])
                                    nc.sync.dma_start(out=bv_sb, in_=bv[:, :])
                                    nc.vector.memset(eps_sb, EPS)
                                    nc.sync.dma_start(
                                        out=wqa_sb,
                                        in_=wqa[:, :].rearrange("(t p) e -> p t e", p=P),
                                    )
                                    nc.sync.dma_start(
                                        out=wka_sb,
                                        in_=wka[:, :].rearrange("(t p) e -> p t e", p=P),
                                    )
                                    nc.sync.dma_start(
                                        out=wv_sb,
                                        in_=wv[:, :].rearrange("(t p) e -> p t e", p=P),
                                    )
                                    nc.sync.dma_start(
                                        out=wzT_sb,
                                        in_=wzT[:, :].rearrange("(t p) e -> p t e", p=P),
                                    )
                                    # prefetch tail-phase inputs now; they
                                    # land long before the C AllReduce
                                    nc.sync.dma_start(
                                        out=xaT_sb,
                                        in_=xaT[:, :].rearrange("(t p) s -> p t s", p=P),
                                    )
                                    if both_id:
                                        nc.sync.dma_start(
                                            out=rows_bc[:, 0, :],
                                            in_=_bcast_row(rows[3:4, :]),
                                        )
                                    else:
                                        for k in range(5):
                                            nc.sync.dma_start(
                                                out=rows_bc[:, k, :],
                                                in_=_bcast_row(rows[k + 1 : k + 2, :]),
                                            )

                                # ===== phase 2: U = G~ @ wqa, overlapping
                                # the chunked AR (psum persists per chunk)
                                with tc.tile_pool(name="pg", bufs=1) as pg:
                                    g_sb = pg.tile([P, NAT, EA], BF16)
                                    nc.vector.memset(g_sb[:, NET, :], 0.0)
                                    for (t0, t1) in G_CHUNKS:
                                        tf = min(t1, NET)  # full 128-row tiles
                                        nc.sync.dma_start(
                                            out=g_sb[:, t0:tf, :],
                                            in_=g_full[t0 * P : tf * P, :]
                                            .rearrange("(t p) e -> p t e", p=P),
                                        )
                                        if t1 > NET:
                                            nc.sync.dma_start(
                                                out=g_sb[0:1, NET, :],
                                                in_=g_full[E : E + 1, :],
                                            )
                                    for (m0, m1) in [(0, 3), (3, 6), (6, 9)]:
                                        pss = {}
                                        for m in range(m0, m1):
                                            for n in range(NH):
                                                pss[m, n] = ps_mm.tile(
                                                    [P, 512], F32, tag="mm",
                                                    name=f"psu_{m}_{n}",
                                                )
                                        for (t0, t1) in G_CHUNKS:
                                            for m in range(m0, m1):
                                                for n in range(NH):
                                                    for k in range(t0, t1):
                                                        nc.tensor.matmul(
                                                            pss[m, n],
                                                            g_sb[:, k, m * P : (m + 1) * P],
                                                            wqa_sb[:, k, n * 512 : (n + 1) * 512],
                                                            start=(k == 0),
                                                            stop=(k == NAT - 1),
                                                        )
                                        for m in range(m0, m1):
                                            for n in range(NH):
                                                nc.vector.tensor_copy(
                                                    out=u_sb[:, m, n * 512 : (n + 1) * 512],
                                                    in_=pss[m, n],
                                                )

                                # ===== phase 3: scores + softmax + A^T =====
                                with tc.tile_pool(name="p3", bufs=3) as p3:
                                    pend = []
                                    for m in range(NET):
                                        pss = [
                                            ps_mm.tile([P, 512], F32, tag="mm",
                                                       name=f"pssc_{n}")
                                            for n in range(NH)
                                        ]
                                        for k in range(NAT):
                                            lhs = wka_sb[:, k, m * P : (m + 1) * P]
                                            for n in range(NH):
                                                nc.tensor.matmul(
                                                    pss[n], lhs,
                                                    u_sb[:, k, n * 512 : (n + 1) * 512],
                                                    start=(k == 0), stop=(k == NAT - 1),
                                                )
                                        mxs = stat.tile([P, NH], F32, tag="mxs")
                                        for n in range(NH):
                                            nc.vector.reduce_max(
                                                out=mxs[:, n : n + 1], in_=pss[n],
                                                axis=mybir.AxisListType.X,
                                            )
                                        mx = stat.tile([P, 1], F32, tag="mx")
                                        nc.vector.tensor_max(
                                            mx, mxs[:, 0:1], mxs[:, 1:2]
                                        )
                                        negmx = stat.tile([P, 1], F32, tag="negmx")
                                        nc.vector.tensor_scalar_mul(negmx, mx, -SCALE)
                                        a_bf = p3.tile([P, E], BF16, tag="abf")
                                        rsums = stat.tile([P, NH], F32, tag="rsums")
                                        for n in range(NH):
                                            nc.scalar.activation(
                                                out=a_bf[:, n * 512 : (n + 1) * 512],
                                                in_=pss[n],
                                                func=mybir.ActivationFunctionType.Exp,
                                                bias=negmx, scale=SCALE,
                                                accum_out=rsums[:, n : n + 1],
                                            )
                                        rsum = stat.tile([P, 1], F32, tag="rsum")
                                        nc.vector.tensor_add(
                                            rsum, rsums[:, 0:1], rsums[:, 1:2]
                                        )
                                        nc.vector.reciprocal(
                                            out=rcp_sb[:, m : m + 1], in_=rsum
                                        )
                                        pend.append((m, a_bf))
                                        if len(pend) > 1:
                                            pm, pa = pend.pop(0)
                                            for fb in range(NET):
                                                pst = ps_tr.tile([P, P], BF16, tag="tr", name="pst")
                                                nc.tensor.transpose(
                                                    pst, pa[:, fb * P : (fb + 1) * P], ident
                                                )
                                                nc.vector.tensor_copy(
                                                    out=at_sb[:, fb, pm * P : (pm + 1) * P],
                                                    in_=pst,
                                                )

                                    for (pm, pa) in pend:
                                        for fb in range(NET):
                                            pst = ps_tr.tile([P, P], BF16, tag="tr", name="pst")
                                            nc.tensor.transpose(
                                                pst, pa[:, fb * P : (fb + 1) * P], ident
                                            )
                                            nc.vector.tensor_copy(
                                                out=at_sb[:, fb, pm * P : (pm + 1) * P],
                                                in_=pst,
                                            )

                            # ===== phase 4a: B = AT.T @ WzT (row-scaled) =====
                            for m in range(NET):
                                mm_loop(
                                    lambda k: at_sb[:, k, m * P : (m + 1) * P],
                                    lambda k, n: wzT_sb[:, k, n * 512 : (n + 1) * 512],
                                    NET,
                                    lambda n, ps: nc.vector.tensor_scalar_mul(
                                        b_sb[:, m, n * 512 : (n + 1) * 512],
                                        ps,
                                        rcp_sb[:, m : m + 1],
                                    ),
                                    ps_mm,
                                )

                        # ===== phase 4b: r + C_h = Wv^T B per row-tile, DMA
                        # out, chunked 8-way AllReduce of [C; r] =====
                        # r = bv^T B + bz/8 -> row E of the AR payload
                        r_sb = stat.tile([1, E], F32, tag="rrow")
                        rbf = p5.tile([1, E], BF16, tag="rbf")
                        for n in range(NH):
                            psr = ps_mm.tile([1, 512], F32, tag="mm", name="psr")
                            for k in range(NET):
                                nc.tensor.matmul(
                                    psr,
                                    bv_sb[:, k : k + 1],
                                    b_sb[:, k, n * 512 : (n + 1) * 512],
                                    start=(k == 0),
                                    stop=(k == NET - 1),
                                )
                            nc.vector.tensor_add(
                                r_sb[:, n * 512 : (n + 1) * 512],
                                psr,
                                bz8_sb[:, n * 512 : (n + 1) * 512],
                            )
                        nc.vector.tensor_copy(out=rbf, in_=r_sb)
                        nc.sync.dma_start(out=c_part[E : E + 1, :], in_=rbf)
                        for m in range(NET):
                            cstg = p5.tile([P, E], BF16, tag="cstg", bufs=3)
                            mm_loop(
                                lambda k: wv_sb[:, k, m * P : (m + 1) * P],
                                lambda k, n: b_sb[:, k, n * 512 : (n + 1) * 512],
                                NET,
                                lambda n, ps: nc.vector.tensor_copy(
                                    out=cstg[:, n * 512 : (n + 1) * 512], in_=ps
                                ),
                                ps_mm,
                            )
                            nc.sync.dma_start(
                                out=c_part[m * P : (m + 1) * P, :], in_=cstg
                            )
                            if m == 3:
                                nc.gpsimd.collective_compute(
                                    "AllReduce",
                                    mybir.AluOpType.add,
                                    replica_groups=rg8,
                                    ins=[c_part[0:512, :]],
                                    outs=[c_full[0:512, :]],
                                )
                        nc.gpsimd.collective_compute(
                            "AllReduce",
                            mybir.AluOpType.add,
                            replica_groups=rg8,
                            ins=[c_part[512 : E + 1, :]],
                            outs=[c_full[512 : E + 1, :]],
                        )

                # ===== phase 5: Opart on own SS rows (chunk-overlapped with
                # the C AllReduce), then LN1+transposes, then FFN+LN2 =====
                with tc.tile_pool(name="pln", bufs=1) as pln, \
                     tc.tile_pool(name="p7", bufs=3) as p7:
                    osb = pln.tile([P, NST, E], BF16)
                    ln1_sb = pln.tile([P, NST, E], F32)
                    l1t_sb = pln.tile([P, NET, SS], BF16)
                    wfT_sb = pln.tile([P, NET, E], BF16)
                    nc.sync.dma_start(
                        out=wfT_sb,
                        in_=wfT[:, :].rearrange("(t p) e -> p t e", p=P),
                    )

                    # residual rows: prefetch all 4 tiles now
                    xsts = []
                    for st in range(NST):
                        xst = p7.tile([P, E], F32, tag="xst", bufs=4)
                        nc.sync.dma_start(out=xst, in_=xs[st * P : (st + 1) * P, :])
                        xsts.append(xst)

                    # AllReduced C chunks -> SBUF; r row broadcast
                    nc.sync.dma_start(
                        out=cs_sb[:, 0:4, :],
                        in_=c_full[0:512, :].rearrange("(t p) e -> p t e", p=P),
                    )
                    nc.sync.dma_start(
                        out=cs_sb[:, 4:8, :],
                        in_=c_full[512:E, :].rearrange("(t p) e -> p t e", p=P),
                    )
                    nc.sync.dma_start(out=rbc_sb, in_=_bcast_row(c_full[E : E + 1, :]))

                    # Opart in two k-passes: pass 1 (AR chunk 1 rows) for
                    # ALL m-tiles into an SBUF accumulator, pass 2 (chunk 2)
                    # adds it back -- the in-order PE queue never blocks
                    # chunk-1 work behind a chunk-2 wait
                    oacc = pln.tile([P, NST, E], F32)
                    for m in range(NST):
                        for n in range(NH):
                            ps1 = ps_mm.tile(
                                [P, 512], F32, tag="mm", name=f"pso1_{m}_{n}"
                            )
                            for k in range(4):
                                nc.tensor.matmul(
                                    ps1,
                                    xaT_sb[:, k, m * P : (m + 1) * P],
                                    cs_sb[:, k, n * 512 : (n + 1) * 512],
                                    start=(k == 0),
                                    stop=(k == 3),
                                )
                            nc.vector.tensor_copy(
                                out=oacc[:, m, n * 512 : (n + 1) * 512], in_=ps1
                            )
                    for m in range(NST):
                        for n in range(NH):
                            ps2 = ps_mm.tile(
                                [P, 512], F32, tag="mm", name=f"pso2_{m}_{n}"
                            )
                            for k in range(4, 8):
                                nc.tensor.matmul(
                                    ps2,
                                    xaT_sb[:, k, m * P : (m + 1) * P],
                                    cs_sb[:, k, n * 512 : (n + 1) * 512],
                                    start=(k == 4),
                                    stop=(k == NET - 1),
                                )
                            tmp = p7.tile([P, 512], F32, tag="otmp")
                            nc.vector.tensor_add(
                                tmp, ps2, oacc[:, m, n * 512 : (n + 1) * 512]
                            )
                            nc.vector.tensor_add(
                                osb[:, m, n * 512 : (n + 1) * 512],
                                tmp,
                                rbc_sb[:, n * 512 : (n + 1) * 512],
                            )

                    def layer_norm(dst, src, r_g, r_b, skip_gb):
                        bst = stat.tile([P, 2, 6], F32, tag="bst")
                        nc.vector.bn_stats(out=bst[:, 0, :], in_=src[:, 0:512])
                        nc.vector.bn_stats(out=bst[:, 1, :], in_=src[:, 512:E])
                        mv = stat.tile([P, 2], F32, tag="mv")
                        nc.vector.bn_aggr(out=mv, in_=bst)
                        sd = stat.tile([P, 1], F32, tag="sd")
                        nc.scalar.activation(
                            out=sd, in_=mv[:, 1:2],
                            func=mybir.ActivationFunctionType.Sqrt, bias=eps_sb[:, :],
                        )
                        rstd = stat.tile([P, 1], F32, tag="rstd")
                        nc.vector.reciprocal(out=rstd, in_=sd)
                        nc.vector.tensor_scalar(
                            out=dst, in0=src, scalar1=mv[:, 0:1], scalar2=rstd,
                            op0=mybir.AluOpType.subtract, op1=mybir.AluOpType.mult,
                        )
                        if not skip_gb:
                            nc.vector.tensor_mul(dst, dst, rows_bc[:, r_g, :])
                            nc.vector.tensor_add(dst, dst, rows_bc[:, r_b, :])

                    # LN1 (+x residual) and transposes for ALL row tiles first
                    # so the FFN/LN2 loop below pipelines PE vs DVE cleanly
                    for st in range(NST):
                        t1 = ln1_sb[:, st, :]
                        ln = p7.tile([P, E], F32, tag="ln")
                        layer_norm(ln, osb[:, st, :], L_G1, L_B1, id_g1b1)
                        nc.vector.tensor_add(t1, ln, xsts[st])
                    for st in range(NST):
                        t1 = ln1_sb[:, st, :]
                        for eb in range(NET):
                            pstf = ps_tr.tile([P, P], F32, tag="tr", name="pstf")
                            nc.tensor.transpose(pstf, t1[:, eb * P : (eb + 1) * P], identf)
                            nc.scalar.activation(
                                out=l1t_sb[:, eb, st * P : (st + 1) * P],
                                in_=pstf,
                                func=mybir.ActivationFunctionType.Copy,
                            )
                    # FFN + LN2 per row tile
                    for st in range(NST):
                        f1 = p7.tile([P, E], F32, tag="f1")
                        mm_loop(
                            lambda k: l1t_sb[:, k, st * P : (st + 1) * P],
                            lambda k, n: wfT_sb[:, k, n * 512 : (n + 1) * 512],
                            NET,
                            lambda n, ps: nc.vector.tensor_add(
                                f1[:, n * 512 : (n + 1) * 512],
                                ps,
                                rows_bc[:, L_BF, n * 512 : (n + 1) * 512],
                            ),
                            ps_mm,
                        )
                        ln2 = p7.tile([P, E], F32, tag="ln2")
                        layer_norm(ln2, f1, L_G2, L_B2, id_g2b2)
                        fo = p7.tile([P, E], F32, tag="ln")
                        nc.vector.tensor_add(fo, ln2, ln1_sb[:, st, :])
                        nc.sync.dma_start(out=out[st * P : (st + 1) * P, :], in_=fo)

    nc.finalize()
    return nc


_NC_CACHE = None


def kernel(**inputs) -> np.ndarray:
    global _NC_CACHE, LAST_RESULT
    x = np.asarray(inputs["x"], np.float32)
    Wq = np.asarray(inputs["Wq"], np.float32)
    bq = np.asarray(inputs["bq"], np.float32)
    Wk = np.asarray(inputs["Wk"], np.float32)
    bk = np.asarray(inputs["bk"], np.float32)
    Wv = np.asarray(inputs["Wv"], np.float32)
    bv = np.asarray(inputs["bv"], np.float32)
    Wz = np.asarray(inputs["Wz"], np.float32)
    bz = np.asarray(inputs["bz"], np.float32)
    g1 = np.asarray(inputs["g1"], np.float32)
    b1 = np.asarray(inputs["b1"], np.float32)
    Wf = np.asarray(inputs["Wf"], np.float32)
    bf_ = np.asarray(inputs["bf"], np.float32)
    g2 = np.asarray(inputs["g2"], np.float32)
    b2 = np.asarray(inputs["b2"], np.float32)

    BF = ml_dtypes.bfloat16
    id_g1b1 = bool(np.all(g1 == 1.0) and np.all(b1 == 0.0))
    id_g2b2 = bool(np.all(g2 == 1.0) and np.all(b2 == 0.0))
    key = (id_g1b1, id_g2b2)
    if _NC_CACHE is None or _NC_CACHE[0] != key:
        _NC_CACHE = (key, build_nc(id_g1b1, id_g2b2))
    nc = _NC_CACHE[1]

    wfT_np = np.ascontiguousarray(Wf.T).astype(BF)
    rows_np = np.ascontiguousarray(
        np.stack([bz / H, g1, b1, bf_, g2, b2]).astype(np.float32)
    )
    pad_w = np.zeros((EA - E - 1, E), np.float32)

    in_maps = []
    for h in range(H):
        own = slice(h * SS, (h + 1) * SS)      # own output shard rows
        xga = x[h * SG : (h + 1) * SG]         # G-partial rows (8-way AR)
        xsa_h = np.concatenate(
            [xga, np.ones((SG, 1), np.float32), np.zeros((SG, EA - E - 1), np.float32)],
            axis=1,
        ).astype(BF)
        xaT_h = np.ascontiguousarray(x[own].T).astype(BF)
        xs_h = np.ascontiguousarray(x[own])
        wqa_h = np.concatenate([Wq[h].T, bq[h][None, :], pad_w], axis=0).astype(BF)
        wka_h = np.concatenate([Wk[h].T, bk[h][None, :], pad_w], axis=0).astype(BF)
        wzT_h = np.ascontiguousarray(Wz[:, h * E : (h + 1) * E].T).astype(BF)
        bv_h = np.ascontiguousarray(bv[h].reshape(NET, P).T).astype(BF)
        in_maps.append(
            {
                "xsa": np.ascontiguousarray(xsa_h),
                "xaT": xaT_h,
                "xs": xs_h,
                "wqa": np.ascontiguousarray(wqa_h),
                "wka": np.ascontiguousarray(wka_h),
                "wv": Wv[h].astype(BF),
                "wzT": wzT_h,
                "wfT": wfT_np,
                "bv": bv_h,
                "rows": rows_np,
            }
        )

    res = run_bass_kernel_spmd(nc, in_maps, list(range(H)))
    LAST_RESULT = res
    out = np.empty((S, E), np.float32)
    for h in range(H):
        out[h * SS : (h + 1) * SS] = res.results[h]["out"]
    return out


# revision 32
# speedup vs baseline: 1.3752x; 1.1449x over previous
"""Distributed Trainium2 kernel for the fused attention-autoencoder layer.

Reference math (per head h):
  Q = x @ Wq_h^T + bq_h ; K = x @ Wk_h^T + bk_h ; V = x @ Wv_h^T + bv_h
  scores = K^T Q / sqrt(E); A = softmax(scores, -1); Zh = V @ A
  O = concat_h(Zh) @ Wz^T + bz ; LN1 = ln(O)*g1+b1 + x
  FN = LN1 @ Wf^T + bf ; out = ln(FN)*g2+b2 + LN1

Restructuring (head h lives on core h):
  With xa = [x | 1] (augmented) and G~ = xa^T xa (symmetric):
    scores_h = Wka_h G~ Wqa_h^T / sqrt(E)  where Wka = [Wk|bk], Wqa = [Wq|bq]
  Each core computes the G~ partial over its own 512 rows and the cores
  8-way AllReduce it in two pieces: rows 0:512 at full (trimmed 1032)
  width, then rows 512:1152 at half width only -- the lower-left block
  of the symmetric G~ is rebuilt on-chip by transposing the AllReduced
  upper-right block, shaving both matmul work and wire bytes.
  A_h = softmax(scores_h). Since the head-concat matmul distributes,
    O = sum_h V_h A_h Wz_h^T = x (sum_h C_h) + 1 (sum_h r_h)^T
  with C_h = Wv_h^T A'_h Wz_h^T and r_h = bv_h^T A'_h Wz_h^T + bz/8, so the
  cross-core exchange is an 8-way AllReduce of the tiny [E+1, E] C/r
  matrix (NOT an S-sized ReduceScatter), shipped in two column halves so
  the first AR flies while the second half computes, and each core then
  runs one small Opart matmul over only its own S/8 rows:
  O_rows = x_rows C + 1 r^T, followed by the LN1/FFN/LN2 pipeline on
  those rows. The host concatenates the 8 contiguous row shards.
"""

import numpy as np
import ml_dtypes

import concourse.bass as bass
import concourse.mybir as mybir
import concourse.tile as tile
from concourse import bacc
from concourse.bass_utils import run_bass_kernel_spmd
from concourse.masks import make_identity

S, E, H = 4096, 1024, 8
P = 128
EA = 1152          # augmented (E + ones col) padded to 9*128
NET = E // P       # 8
NAT = EA // P      # 9
SS = S // H        # 512 rows per core (own contiguous output shard)
NST = SS // P      # 4
SG = 512           # rows per core for the G partial (8-way AR)
NSG = SG // P      # 4
NH = E // 512      # 2 free-dim halves
EPS = 1e-5
SCALE = 1.0 / 32.0  # 1/sqrt(E)

F32 = mybir.dt.float32
BF16 = mybir.dt.bfloat16

LAST_RESULT = None  # test harness reads exec_time_ns off this


def _bcast_row(t: bass.AP) -> bass.AP:
    """[1, n] DRAM row -> partition-broadcast AP."""
    return bass.AP(tensor=t.tensor, offset=t.offset, ap=[[0, P], [1, t.shape[-1]]])


def build_nc(id_g1b1=False, id_g2b2=False):
    nc = bacc.Bacc(num_devices=H)

    xsa = nc.declare_dram_parameter("xsa", [SG, EA], BF16, isOutput=False)
    xaT = nc.declare_dram_parameter("xaT", [E, SS], BF16, isOutput=False)
    xs = nc.declare_dram_parameter("xs", [SS, E], BF16, isOutput=False)
    wqa = nc.declare_dram_parameter("wqa", [EA, E], BF16, isOutput=False)
    wka = nc.declare_dram_parameter("wka", [EA, E], BF16, isOutput=False)
    wv = nc.declare_dram_parameter("wv", [E, E], BF16, isOutput=False)
    wzT = nc.declare_dram_parameter("wzT", [E, E], BF16, isOutput=False)
    wfT = nc.declare_dram_parameter("wfT", [E, E], BF16, isOutput=False)
    bv = nc.declare_dram_parameter("bv", [P, NET], BF16, isOutput=False)
    rows = nc.declare_dram_parameter("rows", [6, E], F32, isOutput=False)
    out = nc.declare_dram_parameter("out", [SS, E], F32, isOutput=True)

    GW = 1032  # G col width: E + ones col + pad 7
    GW2 = GW - 512  # chunk-2 col width (cols 512:1032; left block comes from
    g_part = nc.dram_tensor("g_part", [512, GW], BF16)   # rows 0:512
    g_full = nc.dram_tensor("g_full", [512, GW], BF16, addr_space="Shared")
    g_part2 = nc.dram_tensor("g_part2", [640, GW2], BF16)  # rows 512:1152
    g_full2 = nc.dram_tensor("g_full2", [640, GW2], BF16, addr_space="Shared")
    CH = E + P  # rows per column-half block (E C rows + r row + pad)
    urow = nc.dram_tensor("urow", [1, E], BF16)
    c_part = nc.dram_tensor("c_part", [NH * CH, 512], BF16)
    c_full = nc.dram_tensor("c_full", [NH * CH, 512], BF16, addr_space="Shared")

    rg8 = [list(range(H))]

    both_id = id_g1b1 and id_g2b2
    NROWS = 1 if both_id else 5  # broadcast LN/FFN rows kept in SBUF
    # row indices within rows_bc
    if both_id:
        L_G1 = L_B1 = L_G2 = L_B2 = 0
        L_BF = 0
    else:
        L_G1, L_B1, L_BF, L_G2, L_B2 = range(5)

    def mm_loop(lhs_fn, rhs_fn, nk, evac, ps_pool):
        pss = [
            ps_pool.tile([P, 512], F32, tag="mm", name=f"psmm_{n}") for n in range(NH)
        ]
        for k in range(nk):
            lhs = lhs_fn(k)
            for n in range(NH):
                nc.tensor.matmul(
                    pss[n], lhs, rhs_fn(k, n), start=(k == 0), stop=(k == nk - 1)
                )
        for n in range(NH):
            evac(n, pss[n])

    with tile.TileContext(nc) as tc:
        with (
            tc.tile_pool(name="singles", bufs=1) as singles,
            tc.tile_pool(name="stat", bufs=4) as stat,
            tc.tile_pool(name="ps_mm", bufs=6, space="PSUM") as ps_mm,
            tc.tile_pool(name="ps_tr", bufs=2, space="PSUM") as ps_tr,
        ):
            ident = singles.tile([P, P], BF16)
            identf = singles.tile([P, P], F32)
            bz8_sb = singles.tile([1, E], F32)
            bv_sb = singles.tile([P, NET], BF16)
            rcp_sb = singles.tile([P, NET], F32)
            rbc_sb = singles.tile([P, E], BF16)
            eps_sb = singles.tile([P, 1], F32)
            u_bc = singles.tile([P, E], BF16)
            bk_col = singles.tile([P, NET], BF16)
            bk_f32 = singles.tile([P, NET], F32)
            cs_f32 = singles.tile([P, NET], F32)
            bq_bc = singles.tile([P, E], BF16)

            with tc.tile_pool(name="pc", bufs=1) as pc, \
                 tc.tile_pool(name="p5", bufs=3) as p5:
                cs_sb = pc.tile([P, NET, E], BF16)   # AllReduced sum_h C_h
                xaT_sb = pc.tile([P, NET, SS], BF16)
                rows_bc = pc.tile([P, NROWS, E], F32)
                with tc.tile_pool(name="pwz", bufs=1) as pwz:
                    wv_sb = pwz.tile([P, NET, E], BF16)
                    wzT_sb = pwz.tile([P, NET, E], BF16)
                    with tc.tile_pool(name="pb", bufs=1) as pb:
                        b_sb = pb.tile([P, NET, E], BF16)
                        with tc.tile_pool(name="pat", bufs=1) as pat:
                            at_sb = pat.tile([P, NET, E], BF16)
                            with tc.tile_pool(name="pwqk", bufs=1) as pwqk:
                                wqa_sb = pwqk.tile([P, NAT, E], BF16)
                                wka_sb = pwqk.tile([P, NAT, E], BF16)
                                u_sb = pwqk.tile([P, NAT, E], BF16)
                                with tc.tile_pool(name="pxsa", bufs=1) as pxsa, \
                                     tc.tile_pool(name="p1w", bufs=2) as p1w:
                                    # ===== phase 1: G~ partial (own 512
                                    # rows); 8-way AR: rows 0:512 full width,
                                    # rows 512:1152 right half only =====
                                    xsa_sb = pxsa.tile([P, NSG, EA], BF16)
                                    hs = NSG // 2
                                    nc.sync.dma_start(
                                        out=xsa_sb[:, 0:hs, :],
                                        in_=xsa[0 : hs * P, :]
                                        .rearrange("(t p) e -> p t e", p=P),
                                    )
                                    nc.sync.dma_start(
                                        out=xsa_sb[:, hs:NSG, :],
                                        in_=xsa[hs * P : SG, :]
                                        .rearrange("(t p) e -> p t e", p=P),
                                    )
                                    for m in range(4):
                                        gp = p1w.tile([P, GW], BF16, tag="gp")
                                        for (n0, nw) in [
                                            (0, 512), (512, 512), (1024, GW - 1024)
                                        ]:
                                            ps = ps_mm.tile(
                                                [P, nw], F32, tag="mm", name="psg"
                                            )
                                            for k in range(NSG):
                                                nc.tensor.matmul(
                                                    ps,
                                                    xsa_sb[:, k, m * P : (m + 1) * P],
                                                    xsa_sb[:, k, n0 : n0 + nw],
                                                    start=(k == 0),
                                                    stop=(k == NSG - 1),
                                                )
                                            nc.vector.tensor_copy(
                                                out=gp[:, n0 : n0 + nw], in_=ps
                                            )
                                        nc.sync.dma_start(
                                            out=g_part[m * P : (m + 1) * P, :], in_=gp
                                        )
                                    nc.gpsimd.collective_compute(
                                        "AllReduce",
                                        mybir.AluOpType.add,
                                        replica_groups=rg8,
                                        ins=[g_part[:, :]],
                                        outs=[g_full[:, :]],
                                    )
                                    # rows 512:1152, cols 512:1032 only -- the
                                    # left block is chunk 1 transposed (G sym)
                                    for m in range(4, 9):
                                        gp2 = p1w.tile([P, GW2], BF16, tag="gp2")
                                        for (n0, nw) in [(512, 512), (1024, GW - 1024)]:
                                            ps = ps_mm.tile(
                                                [P, nw], F32, tag="mm", name="psg"
                                            )
                                            for k in range(NSG):
                                                nc.tensor.matmul(
                                                    ps,
                                                    xsa_sb[:, k, m * P : (m + 1) * P],
                                                    xsa_sb[:, k, n0 : n0 + nw],
                                                    start=(k == 0),
                                                    stop=(k == NSG - 1),
                                                )
                                            nc.vector.tensor_copy(
                                                out=gp2[:, n0 - 512 : n0 - 512 + nw],
                                                in_=ps,
                                            )
                                        nc.sync.dma_start(
                                            out=g_part2[
                                                (m - 4) * P : (m - 3) * P, :
                                            ],
                                            in_=gp2,
                                        )
                                    nc.gpsimd.collective_compute(
                                        "AllReduce",
                                        mybir.AluOpType.add,
                                        replica_groups=rg8,
                                        ins=[g_part2[0:513, :]],
                                        outs=[g_full2[0:513, :]],
                                    )

                                    # ---- constants / weights (emitted after
                                    # the collectives: G path wins DMA prio) ----
                                    make_identity(nc, ident)
                                    make_identity(nc, identf)
                                    nc.sync.dma_start(out=bz8_sb, in_=rows[0:1, :])
                                    nc.sync.dma_start(out=bv_sb, in_=bv[:, :])
                                    nc.vector.memset(eps_sb, EPS)
                                    nc.sync.dma_start(
                                        out=wqa_sb,
                                        in_=wqa[:, :].rearrange("(t p) e -> p t e", p=P),
                                    )
                                    nc.sync.dma_start(
                                        out=wka_sb,
                                        in_=wka[:, :].rearrange("(t p) e -> p t e", p=P),
                                    )
                                    nc.sync.dma_start(
                                        out=wv_sb,
                                        in_=wv[:, :].rearrange("(t p) e -> p t e", p=P),
                                    )
                                    nc.sync.dma_start(
                                        out=wzT_sb,
                                        in_=wzT[:, :].rearrange("(t p) e -> p t e", p=P),
                                    )
                                    nc.sync.dma_start(
                                        out=bq_bc, in_=_bcast_row(wqa[E : E + 1, :])
                                    )
                                    bkr = wka[E : E + 1, :]
                                    nc.sync.dma_start(
                                        out=bk_col,
                                        in_=bass.AP(
                                            tensor=bkr.tensor,
                                            offset=bkr.offset,
                                            ap=[[1, P], [P, NET]],
                                        ),
                                    )
                                    nc.vector.tensor_copy(
                                        out=bk_f32, in_=bk_col
                                    )
                                    # prefetch tail-phase inputs now; they
                                    # land long before the C AllReduce
                                    nc.sync.dma_start(
                                        out=xaT_sb,
                                        in_=xaT[:, :].rearrange("(t p) s -> p t s", p=P),
                                    )
                                    if both_id:
                                        nc.sync.dma_start(
                                            out=rows_bc[:, 0, :],
                                            in_=_bcast_row(rows[3:4, :]),
                                        )
                                    else:
                                        for k in range(5):
                                            nc.sync.dma_start(
                                                out=rows_bc[:, k, :],
                                                in_=_bcast_row(rows[k + 1 : k + 2, :]),
                                            )

                                # ===== phase 2: U = G~ @ wqa, overlapping
                                # the chunked AR (psum persists per chunk)
                                with tc.tile_pool(name="pg", bufs=1) as pg:
                                    g_sb = pg.tile([P, NAT, EA], BF16)
                                    nc.vector.memset(g_sb[:, NET, :], 0.0)
                                    nc.vector.memset(g_sb[:, :, GW:EA], 0.0)
                                    nc.sync.dma_start(
                                        out=g_sb[:, 0:4, 0:GW],
                                        in_=g_full[:, :]
                                        .rearrange("(t p) e -> p t e", p=P),
                                    )
                                    def emit_g_transposes():
                                        # lower-left block of G = transpose of
                                        # the upper-right block (AR chunk 1)
                                        for i in range(5):
                                            ct = 512 + i * P
                                            for j in range(4):
                                                pst = ps_tr.tile(
                                                    [P, P], BF16, tag="tr", name="pstg"
                                                )
                                                nc.tensor.transpose(
                                                    pst,
                                                    g_sb[:, j, ct : ct + P],
                                                    ident,
                                                )
                                                nc.vector.tensor_copy(
                                                    out=g_sb[:, 4 + i, j * P : (j + 1) * P],
                                                    in_=pst,
                                                )
                                    nc.sync.dma_start(
                                        out=g_sb[:, 4:8, 512:GW],
                                        in_=g_full2[0:512, :]
                                        .rearrange("(t p) e -> p t e", p=P),
                                    )
                                    nc.sync.dma_start(
                                        out=g_sb[0:1, NET, 512:GW],
                                        in_=g_full2[512:513, :],
                                    )
                                    nc.vector.tensor_copy(
                                        out=cs_f32, in_=g_sb[:, 0:NET, 1024:1025]
                                    )
                                    for (m0, m1) in [(0, 3), (3, 6), (6, 9)]:
                                        pss = {}
                                        for m in range(m0, m1):
                                            for n in range(NH):
                                                pss[m, n] = ps_mm.tile(
                                                    [P, 512], F32, tag="mm",
                                                    name=f"psu_{m}_{n}",
                                                )
                                        for (t0, t1) in [(0, 4), (4, 8)]:
                                            for m in range(m0, m1):
                                                for n in range(NH):
                                                    for k in range(t0, t1):
                                                        nc.tensor.matmul(
                                                            pss[m, n],
                                                            g_sb[:, k, m * P : (m + 1) * P],
                                                            wqa_sb[:, k, n * 512 : (n + 1) * 512],
                                                            start=(k == 0),
                                                            stop=(k == 7),
                                                        )
                                            if (m0, t0) == (0, 0):
                                                emit_g_transposes()
                                        for m in range(m0, m1):
                                            for n in range(NH):
                                                us = u_sb[:, m, n * 512 : (n + 1) * 512]
                                                if m < NET:
                                                    # + cs[m-rows] x bq[n] (the
                                                    # dropped G-row-1024 term)
                                                    uc = pg.tile(
                                                        [P, 512], F32,
                                                        tag="uc", bufs=2,
                                                    )
                                                    nc.vector.tensor_scalar_mul(
                                                        uc,
                                                        bq_bc[:, n * 512 : (n + 1) * 512],
                                                        cs_f32[:, m : m + 1],
                                                    )
                                                    nc.vector.tensor_add(
                                                        us, pss[m, n], uc
                                                    )
                                                else:
                                                    # U row 1024 += S*bq
                                                    nc.vector.tensor_copy(
                                                        out=us, in_=pss[m, n]
                                                    )
                                                    u8 = u_sb[0:1, NET, n * 512 : (n + 1) * 512]
                                                    uc8 = pg.tile(
                                                        [1, 512], F32,
                                                        tag="uc8", bufs=2,
                                                    )
                                                    nc.vector.tensor_scalar_mul(
                                                        uc8,
                                                        bq_bc[0:1, n * 512 : (n + 1) * 512],
                                                        float(S),
                                                    )
                                                    nc.vector.tensor_add(u8, u8, uc8)

                                    nc.sync.dma_start(
                                        out=urow[:, :], in_=u_sb[0:1, NET, :]
                                    )
                                    nc.sync.dma_start(
                                        out=u_bc, in_=_bcast_row(urow[0:1, :])
                                    )

                                # ===== phase 3: scores + softmax + A^T =====
                                with tc.tile_pool(name="p3", bufs=3) as p3:
                                    pend = []
                                    for m in range(NET):
                                        pss = [
                                            ps_mm.tile([P, 512], F32, tag="mm",
                                                       name=f"pssc_{n}")
                                            for n in range(NH)
                                        ]
                                        for k in range(NET):
                                            lhs = wka_sb[:, k, m * P : (m + 1) * P]
                                            for n in range(NH):
                                                nc.tensor.matmul(
                                                    pss[n], lhs,
                                                    u_sb[:, k, n * 512 : (n + 1) * 512],
                                                    start=(k == 0), stop=(k == 7),
                                                )
                                        # + bk[m-rows] x U[1024,:] (the dropped
                                        # wka-row-1024 contraction term)
                                        scr = p3.tile([P, E], F32, tag="scr")
                                        for n in range(NH):
                                            sct = p3.tile([P, 512], F32, tag="sct")
                                            nc.vector.tensor_scalar_mul(
                                                sct,
                                                u_bc[:, n * 512 : (n + 1) * 512],
                                                bk_f32[:, m : m + 1],
                                            )
                                            nc.vector.tensor_add(
                                                scr[:, n * 512 : (n + 1) * 512],
                                                pss[n],
                                                sct,
                                            )
                                        mxs = stat.tile([P, NH], F32, tag="mxs")
                                        for n in range(NH):
                                            nc.vector.reduce_max(
                                                out=mxs[:, n : n + 1],
                                                in_=scr[:, n * 512 : (n + 1) * 512],
                                                axis=mybir.AxisListType.X,
                                            )
                                        mx = stat.tile([P, 1], F32, tag="mx")
                                        nc.vector.tensor_max(
                                            mx, mxs[:, 0:1], mxs[:, 1:2]
                                        )
                                        negmx = stat.tile([P, 1], F32, tag="negmx")
                                        nc.vector.tensor_scalar_mul(negmx, mx, -SCALE)
                                        a_bf = p3.tile([P, E], BF16, tag="abf")
                                        rsums = stat.tile([P, NH], F32, tag="rsums")
                                        for n in range(NH):
                                            nc.scalar.activation(
                                                out=a_bf[:, n * 512 : (n + 1) * 512],
                                                in_=scr[:, n * 512 : (n + 1) * 512],
                                                func=mybir.ActivationFunctionType.Exp,
                                                bias=negmx, scale=SCALE,
                                                accum_out=rsums[:, n : n + 1],
                                            )
                                        rsum = stat.tile([P, 1], F32, tag="rsum")
                                        nc.vector.tensor_add(
                                            rsum, rsums[:, 0:1], rsums[:, 1:2]
                                        )
                                        nc.vector.reciprocal(
                                            out=rcp_sb[:, m : m + 1], in_=rsum
                                        )
                                        pend.append((m, a_bf))
                                        if len(pend) > 1:
                                            pm, pa = pend.pop(0)
                                            for fb in range(NET):
                                                pst = ps_tr.tile([P, P], BF16, tag="tr", name="pst")
                                                nc.tensor.transpose(
                                                    pst, pa[:, fb * P : (fb + 1) * P], ident
                                                )
                                                nc.vector.tensor_copy(
                                                    out=at_sb[:, fb, pm * P : (pm + 1) * P],
                                                    in_=pst,
                                                )

                                    for (pm, pa) in pend:
                                        for fb in range(NET):
                                            pst = ps_tr.tile([P, P], BF16, tag="tr", name="pst")
                                            nc.tensor.transpose(
                                                pst, pa[:, fb * P : (fb + 1) * P], ident
                                            )
                                            nc.vector.tensor_copy(
                                                out=at_sb[:, fb, pm * P : (pm + 1) * P],
                                                in_=pst,
                                            )

                            pass  # phases 4a/4b below (column-halved)
                        # ===== phase 4: per column half: B half, r half,
                        # C half, then a strided AllReduce of that half of
                        # [C; r] -- the first AR flies while the second
                        # half computes, and Opart consumes per half =====
                        for nh in range(NH):
                            n0 = nh * 512
                            for m in range(NET):
                                psb = ps_mm.tile([P, 512], F32, tag="mm",
                                                 name=f"psb_{m}")
                                for k in range(NET):
                                    nc.tensor.matmul(
                                        psb,
                                        at_sb[:, k, m * P : (m + 1) * P],
                                        wzT_sb[:, k, n0 : n0 + 512],
                                        start=(k == 0), stop=(k == NET - 1),
                                    )
                                nc.vector.tensor_scalar_mul(
                                    b_sb[:, m, n0 : n0 + 512],
                                    psb,
                                    rcp_sb[:, m : m + 1],
                                )
                            psr = ps_mm.tile([1, 512], F32, tag="mm", name="psr")
                            for k in range(NET):
                                nc.tensor.matmul(
                                    psr,
                                    bv_sb[:, k : k + 1],
                                    b_sb[:, k, n0 : n0 + 512],
                                    start=(k == 0), stop=(k == NET - 1),
                                )
                            rbf = p5.tile([1, 512], BF16, tag="rbf")
                            nc.vector.tensor_add(
                                rbf, psr, bz8_sb[:, n0 : n0 + 512]
                            )
                            nc.sync.dma_start(
                                out=c_part[E : E + 1, n0 : n0 + 512], in_=rbf
                            )
                            for m in range(NET):
                                psc = ps_mm.tile([P, 512], F32, tag="mm",
                                                 name=f"psc_{m}")
                                for k in range(NET):
                                    nc.tensor.matmul(
                                        psc,
                                        wv_sb[:, k, m * P : (m + 1) * P],
                                        b_sb[:, k, n0 : n0 + 512],
                                        start=(k == 0), stop=(k == NET - 1),
                                    )
                                cstg = p5.tile([P, 512], BF16, tag="cstg", bufs=3)
                                nc.vector.tensor_copy(out=cstg, in_=psc)
                                nc.sync.dma_start(
                                    out=c_part[m * P : (m + 1) * P, n0 : n0 + 512],
                                    in_=cstg,
                                )
                            nc.gpsimd.collective_compute(
                                "AllReduce",
                                mybir.AluOpType.add,
                                replica_groups=rg8,
                                ins=[c_part[0 : E + 1, n0 : n0 + 512]],
                                outs=[c_full[0 : E + 1, n0 : n0 + 512]],
                            )

                # ===== phase 5: Opart on own SS rows (chunk-overlapped with
                # the C AllReduce), then LN1+transposes, then FFN+LN2 =====
                with tc.tile_pool(name="pln", bufs=1) as pln, \
                     tc.tile_pool(name="p7", bufs=3) as p7:
                    osb = pln.tile([P, NST, E], BF16)
                    ln1_sb = pln.tile([P, NST, E], F32)
                    l1t_sb = pln.tile([P, NET, SS], BF16)
                    wfT_sb = pln.tile([P, NET, E], BF16)
                    nc.sync.dma_start(
                        out=wfT_sb,
                        in_=wfT[:, :].rearrange("(t p) e -> p t e", p=P),
                    )

                    # residual rows: prefetch all 4 tiles now
                    xsts = []
                    for st in range(NST):
                        xst = p7.tile([P, E], BF16, tag="xst", bufs=4)
                        nc.sync.dma_start(out=xst, in_=xs[st * P : (st + 1) * P, :])
                        xsts.append(xst)


                    # Opart per AllReduced column half: each half's psums
                    # complete as soon as that half's AR lands
                    for nh in range(NH):
                        n0 = nh * 512
                        nc.sync.dma_start(
                            out=cs_sb[:, 0:NET, n0 : n0 + 512],
                            in_=c_full[nh * CH : nh * CH + E, :]
                            .rearrange("(t p) e -> p t e", p=P),
                        )
                        nc.sync.dma_start(
                            out=rbc_sb[:, n0 : n0 + 512],
                            in_=_bcast_row(c_full[nh * CH + E : nh * CH + E + 1, :]),
                        )
                        for m in range(NST):
                            pso = ps_mm.tile(
                                [P, 512], F32, tag="mm", name=f"pso_{m}"
                            )
                            for k in range(NET):
                                nc.tensor.matmul(
                                    pso,
                                    xaT_sb[:, k, m * P : (m + 1) * P],
                                    cs_sb[:, k, n0 : n0 + 512],
                                    start=(k == 0),
                                    stop=(k == NET - 1),
                                )
                            nc.vector.tensor_add(
                                osb[:, m, n0 : n0 + 512],
                                pso,
                                rbc_sb[:, n0 : n0 + 512],
                            )

                    def layer_norm(dst, src, r_g, r_b, skip_gb):
                        bst = stat.tile([P, 2, 6], F32, tag="bst")
                        nc.vector.bn_stats(out=bst[:, 0, :], in_=src[:, 0:512])
                        nc.vector.bn_stats(out=bst[:, 1, :], in_=src[:, 512:E])
                        mv = stat.tile([P, 2], F32, tag="mv")
                        nc.vector.bn_aggr(out=mv, in_=bst)
                        sd = stat.tile([P, 1], F32, tag="sd")
                        nc.scalar.activation(
                            out=sd, in_=mv[:, 1:2],
                            func=mybir.ActivationFunctionType.Sqrt, bias=eps_sb[:, :],
                        )
                        rstd = stat.tile([P, 1], F32, tag="rstd")
                        nc.vector.reciprocal(out=rstd, in_=sd)
                        nc.vector.tensor_scalar(
                            out=dst, in0=src, scalar1=mv[:, 0:1], scalar2=rstd,
                            op0=mybir.AluOpType.subtract, op1=mybir.AluOpType.mult,
                        )
                        if not skip_gb:
                            nc.vector.tensor_mul(dst, dst, rows_bc[:, r_g, :])
                            nc.vector.tensor_add(dst, dst, rows_bc[:, r_b, :])

                    # LN1 (+x residual) then transposes, interleaved per
                    # row tile so PE and DVE ping-pong without long stalls
                    for st in range(NST):
                        t1 = ln1_sb[:, st, :]
                        ln = p7.tile([P, E], F32, tag="ln")
                        layer_norm(ln, osb[:, st, :], L_G1, L_B1, id_g1b1)
                        nc.vector.tensor_add(t1, ln, xsts[st])
                        for eb in range(NET):
                            pstf = ps_tr.tile([P, P], F32, tag="tr", name="pstf")
                            nc.tensor.transpose(pstf, t1[:, eb * P : (eb + 1) * P], identf)
                            nc.scalar.activation(
                                out=l1t_sb[:, eb, st * P : (st + 1) * P],
                                in_=pstf,
                                func=mybir.ActivationFunctionType.Copy,
                            )
                    # FFN + LN2 per row tile
                    for st in range(NST):
                        f1 = p7.tile([P, E], F32, tag="f1")
                        mm_loop(
                            lambda k: l1t_sb[:, k, st * P : (st + 1) * P],
                            lambda k, n: wfT_sb[:, k, n * 512 : (n + 1) * 512],
                            NET,
                            lambda n, ps: nc.vector.tensor_add(
                                f1[:, n * 512 : (n + 1) * 512],
                                ps,
                                rows_bc[:, L_BF, n * 512 : (n + 1) * 512],
                            ),
                            ps_mm,
                        )
                        ln2 = p7.tile([P, E], F32, tag="ln2")
                        layer_norm(ln2, f1, L_G2, L_B2, id_g2b2)
                        fo = p7.tile([P, E], F32, tag="ln")
                        nc.vector.tensor_add(fo, ln2, ln1_sb[:, st, :])
                        nc.sync.dma_start(out=out[st * P : (st + 1) * P, :], in_=fo)

    nc.finalize()
    return nc


_NC_CACHE = None


def kernel(**inputs) -> np.ndarray:
    global _NC_CACHE, LAST_RESULT
    x = np.asarray(inputs["x"], np.float32)
    Wq = np.asarray(inputs["Wq"], np.float32)
    bq = np.asarray(inputs["bq"], np.float32)
    Wk = np.asarray(inputs["Wk"], np.float32)
    bk = np.asarray(inputs["bk"], np.float32)
    Wv = np.asarray(inputs["Wv"], np.float32)
    bv = np.asarray(inputs["bv"], np.float32)
    Wz = np.asarray(inputs["Wz"], np.float32)
    bz = np.asarray(inputs["bz"], np.float32)
    g1 = np.asarray(inputs["g1"], np.float32)
    b1 = np.asarray(inputs["b1"], np.float32)
    Wf = np.asarray(inputs["Wf"], np.float32)
    bf_ = np.asarray(inputs["bf"], np.float32)
    g2 = np.asarray(inputs["g2"], np.float32)
    b2 = np.asarray(inputs["b2"], np.float32)

    BF = ml_dtypes.bfloat16
    id_g1b1 = bool(np.all(g1 == 1.0) and np.all(b1 == 0.0))
    id_g2b2 = bool(np.all(g2 == 1.0) and np.all(b2 == 0.0))
    key = (id_g1b1, id_g2b2)
    if _NC_CACHE is None or _NC_CACHE[0] != key:
        _NC_CACHE = (key, build_nc(id_g1b1, id_g2b2))
    nc = _NC_CACHE[1]

    wfT_np = np.ascontiguousarray(Wf.T).astype(BF)
    rows_np = np.ascontiguousarray(
        np.stack([bz / H, g1, b1, bf_, g2, b2]).astype(np.float32)
    )
    pad_w = np.zeros((EA - E - 1, E), np.float32)

    in_maps = []
    for h in range(H):
        own = slice(h * SS, (h + 1) * SS)      # own output shard rows
        xga = x[h * SG : (h + 1) * SG]         # G-partial rows (8-way AR)
        xsa_h = np.concatenate(
            [xga, np.ones((SG, 1), np.float32), np.zeros((SG, EA - E - 1), np.float32)],
            axis=1,
        ).astype(BF)
        xaT_h = np.ascontiguousarray(x[own].T).astype(BF)
        xs_h = np.ascontiguousarray(x[own]).astype(BF)
        wqa_h = np.concatenate([Wq[h].T, bq[h][None, :], pad_w], axis=0).astype(BF)
        wka_h = np.concatenate([Wk[h].T, bk[h][None, :], pad_w], axis=0).astype(BF)
        wzT_h = np.ascontiguousarray(Wz[:, h * E : (h + 1) * E].T).astype(BF)
        bv_h = np.ascontiguousarray(bv[h].reshape(NET, P).T).astype(BF)
        in_maps.append(
            {
                "xsa": np.ascontiguousarray(xsa_h),
                "xaT": xaT_h,
                "xs": xs_h,
                "wqa": np.ascontiguousarray(wqa_h),
                "wka": np.ascontiguousarray(wka_h),
                "wv": Wv[h].astype(BF),
                "wzT": wzT_h,
                "wfT": wfT_np,
                "bv": bv_h,
                "rows": rows_np,
            }
        )

    res = run_bass_kernel_spmd(nc, in_maps, list(range(H)))
    LAST_RESULT = res
    out = np.empty((S, E), np.float32)
    for h in range(H):
        out[h * SS : (h + 1) * SS] = res.results[h]["out"]
    return out
